# revision 11
# baseline (speedup 1.0000x reference)
"""GPT self-attention (B=4, S=2048, D=1024, H=16) on 8 NeuronCores.

Sharding: core c = (batch b = c//2, head-group g = c%2 of 8 heads).
Each core computes q/k/v projections for its 8 heads, causal attention,
and a partial output projection (rows of w_dense for its heads).
Host sums the two partials per batch (tensor-parallel unshard) + bias.

Schedule: heads are processed in pairs (2j, 2j+1) sharing one key-tile
loop, so the two heads' score matmuls (contraction 64, base partitions
0/64) run concurrently in disjoint PE row groups.  Projection GEMMs are
chopped into ~1.7us work units and drip-fed into the attention phases'
ACT-bound gaps.  Softmax normalization is deferred per pair: rowsums
(ones-column of V) are DMA-collected into a [16, S] tile, inverted with
one 2-lane approx-reciprocal per pair, broadcast on GpSimd, and applied
with one multiply per 64-row block.
"""

import numpy as np
import ml_dtypes

import concourse.bass as bass
import concourse.mybir as mybir
import concourse.tile as tile
from concourse import bacc
from concourse import bass_utils

B, S, D, H = 4, 2048, 1024, 16
HD = D // H          # 64
NCORES = 8
GH = 8               # heads per core (group)
DG = GH * HD         # 512 dims per group
P = 128
NKT = S // P         # 16 key tiles
NJ = DG // P         # 4 partition-tiles of group dims
NKD = D // P         # 8 contraction tiles for projections
CH = 512             # psum chunk (one bank of f32)
HW = 1024            # q-half width
NPAIR = GH // 2      # 4 head pairs

BF16 = mybir.dt.bfloat16
F32 = mybir.dt.float32
NPBF16 = ml_dtypes.bfloat16

_COMPILED = None


def _build_body(tc, aps, dbg=None):
    nc = tc.nc
    xT = aps["xT"].rearrange("(k p) s -> p k s", p=P)      # [128, 8, 2048]
    wq = aps["wq"].rearrange("(k p) m -> p k m", p=P)      # [128, 8, 512]
    wk = aps["wk"].rearrange("(k p) m -> p k m", p=P)
    wv = aps["wv"].rearrange("(k p) m -> p k m", p=P)
    wd = aps["wd"].rearrange("(j p) n -> p j n", p=P)      # [128, 4, 1024]
    maskin = aps["mask"]                                   # [128, 128] bf16
    outp = aps["outp"]                                     # [2048, 1024] f32

    Exp = mybir.ActivationFunctionType.Exp

    with (
        tc.tile_pool(name="const", bufs=1) as cpool,
        tc.tile_pool(name="pts", bufs=4) as ppool,
        tc.tile_pool(name="c64", bufs=3) as cstg,
        tc.tile_pool(name="rsg", bufs=2) as rstg,
        tc.tile_pool(name="bc", bufs=2) as bcp,
        tc.tile_pool(name="r0", bufs=2) as r0p,
        tc.tile_pool(name="ost", bufs=2) as ostg,
        tc.tile_pool(name="pssc", bufs=2, space=bass.MemorySpace.PSUM) as psc,
        tc.tile_pool(name="psctx", bufs=2, space=bass.MemorySpace.PSUM) as pcx,
    ):
        # ---- persistent SBUF tensors ----
        xT_t = cpool.tile([P, NKD, S], BF16, tag="xT")
        wq_t = cpool.tile([P, NKD, DG], BF16, tag="wq")
        wk_t = cpool.tile([P, NKD, DG], BF16, tag="wk")
        wv_t = cpool.tile([P, NKD, DG], BF16, tag="wv")
        wd_t = cpool.tile([P, NJ, D], BF16, tag="wd")
        mask_t = cpool.tile([P, P], BF16, tag="mask")
        qT_t = cpool.tile([P, NJ, S], BF16, tag="qT")      # [dim, s]
        kT_t = cpool.tile([P, NJ, S], BF16, tag="kT")
        # v_aug: per s-tile, per head: 64 v-dims + ones column (65 wide)
        v_t = cpool.tile([P, NKT, GH * (HD + 1)], BF16, tag="v")
        ctxT_t = cpool.tile([P, NJ, S], BF16, tag="ctxT")  # normalized ctx^T

        nc.sync.dma_start(wq_t[:], wq)
        nc.sync.dma_start(wk_t[:], wk)
        for kt in range(NKD):
            nc.sync.dma_start(xT_t[:, kt, :], xT[:, kt, :])
        nc.sync.dma_start(wv_t[:], wv)
        nc.sync.dma_start(mask_t[:], maskin)
        nc.sync.dma_start(wd_t[:], wd)
        # ones columns of v_aug
        v_heads = v_t.rearrange("p t (h c) -> p t h c", c=HD + 1)
        nc.vector.memset(v_heads[:, :, :, HD:], 1.0)

        # ---- projection work units (~1.7us of PE each) ----
        def qk_unit(dst, w, j, n0):
            def emit():
                ps = psc.tile([P, 2 * CH], F32, tag="sc")
                for sub in range(2):
                    for kt in range(NKD):
                        nc.tensor.matmul(
                            ps[:, sub * CH:(sub + 1) * CH],
                            w[:, kt, j * P:(j + 1) * P],
                            xT_t[:, kt, n0 + sub * CH:n0 + (sub + 1) * CH],
                            start=(kt == 0), stop=(kt == NKD - 1),
                        )
                nc.vector.tensor_copy(dst[:, j, n0:n0 + 2 * CH], ps[:])
            return emit

        def v_unit(st):
            def emit():
                ps = psc.tile([P, 2 * CH], F32, tag="sc")
                for sub in range(2):
                    for kt in range(NKD):
                        nc.tensor.matmul(
                            ps[:, sub * CH:(sub + 1) * CH],
                            xT_t[:, kt, (st + sub) * P:(st + sub + 1) * P],
                            wv_t[:, kt, :],
                            start=(kt == 0), stop=(kt == NKD - 1),
                        )
                for sub in range(2):
                    nc.vector.tensor_copy(
                        v_heads[:, st + sub, :, 0:HD],
                        ps[:, sub * CH:(sub + 1) * CH]
                        .rearrange("p (h c) -> p h c", c=HD)[:],
                    )
            return emit

        def out_unit(st):
            def emit():
                ps = psc.tile([P, 2 * CH], F32, tag="sc")
                for sub in range(2):
                    for j in range(NJ):
                        nc.tensor.matmul(
                            ps[:, sub * CH:(sub + 1) * CH],
                            ctxT_t[:, j, st * P:(st + 1) * P],
                            wd_t[:, j, sub * CH:(sub + 1) * CH],
                            start=(j == 0), stop=(j == NJ - 1),
                        )
                ost = ostg.tile([P, 2 * CH], F32, tag="ost")
                nc.vector.tensor_copy(ost[:], ps[:])
                nc.sync.dma_start(outp[st * P:(st + 1) * P, :], ost[:])
            return emit

        queue = []

        def fill(n=1):
            for _ in range(n):
                if queue:
                    queue.pop(0)()

        # ---- one head pair (2j, 2j+1) over one q-half ----
        def attention_pair(j, half):
            lo, hi = HW * half, HW * (half + 1)
            hE, hO = 2 * j, 2 * j + 1
            ctxE = pcx.tile([HD + 1, HW], F32, tag="ctx")
            ctxO = pcx.tile([HD + 1, HW], F32, tag="ctx")
            nkt = (half + 1) * (NKT // 2)

            def emit_ctx(kt, q0, pe, po):
                chunks = [(max(q0, CH * m), CH * (m + 1))
                          for m in range(q0 // CH, hi // CH)]
                for h, ctxp, pts in ((hE, ctxE, pe), (hO, ctxO, po)):
                    for c0, c1 in chunks:
                        nc.tensor.matmul(
                            ctxp[:, c0 - lo:c1 - lo],
                            v_t[:, kt, h * (HD + 1):(h + 1) * (HD + 1)],
                            pts[:, c0 - q0:c1 - q0],
                            start=(kt == 0), stop=(kt == nkt - 1),
                            skip_group_check=True,
                        )

            pend = None
            for kt in range(nkt):
                q0 = max(P * kt, lo)
                width = hi - q0
                pe = ppool.tile([P, HW], BF16, tag="pts")
                po = ppool.tile([P, HW], BF16, tag="pts")
                for pb, pts in ((0, pe), (64, po)):
                    sps = psc.tile([P, 2 * CH], F32, tag="sc")
                    for c in range(0, width, CH):
                        cw = min(CH, width - c)
                        nc.tensor.matmul(
                            sps[:, c:c + cw],
                            kT_t[pb:pb + HD, j, P * kt:P * (kt + 1)],
                            qT_t[pb:pb + HD, j, q0 + c:q0 + c + cw],
                            start=True, stop=True,
                        )
                    nc.scalar.activation(
                        pts[:, 0:width], sps[:, 0:width], Exp,
                        scale=1.0 / np.sqrt(HD),
                    )
                    if q0 == P * kt:  # diagonal tile: causal mask
                        nc.vector.tensor_mul(pts[:, 0:P], pts[:, 0:P], mask_t[:])
                if kt in (2, 5):
                    fill(2)
                elif kt in (8, 11, 14):
                    fill(1)
                if pend is not None:
                    emit_ctx(*pend)
                pend = (kt, q0, pe, po)
            emit_ctx(*pend)
            # ---- drain psum fast: ctx rows on DVE (bf16), rowsum on ACT ----
            work = []
            for h, ctxp, pb in ((hE, ctxE, 0), (hO, ctxO, 64)):
                c64 = cstg.tile([HD, HW], BF16, tag="c64")
                nc.vector.tensor_copy(c64[:], ctxp[0:HD, :])
                rsg = rstg.tile([HD + 1, HW], F32, tag="rsg")
                nc.scalar.copy(rsg[HD:HD + 1, :], ctxp[HD:HD + 1, :])
                work.append((c64, rsg, pb))
            # ---- 1/rowsum -> broadcast -> normalize (off critical path) ----
            for c64, rsg, pb in work:
                r0 = r0p.tile([1, HW], F32, tag="r0")
                nc.sync.dma_start(r0[:], rsg[HD:HD + 1, :])
                nc.vector.reciprocal_approx_fast(r0[:], r0[:])
                bc = bcp.tile([HD, HW], F32, tag="bc")
                nc.gpsimd.partition_broadcast(bc[:], r0[:])
                if pb == 0:
                    nc.vector.tensor_mul(
                        ctxT_t[0:HD, j, lo:hi], c64[:], bc[:])
                else:
                    nc.vector.tensor_mul(c64[:], c64[:], bc[:])
                    nc.sync.dma_start(ctxT_t[pb:pb + P // 2, j, lo:hi], c64[:])

        # ---- PE warmup: junk matmuls on wq while xT loads, to flip the
        # HAM clock gate to 8/8 before the real work arrives ----
        junk = psc.tile([P, 2 * CH], F32, tag="sc")
        for i in range(24):
            nc.tensor.matmul(
                junk[:, 0:CH], wq_t[:, 0, 0:P], wq_t[:, i % NKD, 0:CH],
                start=True, stop=True, skip_group_check=True,
            )

        # ---- upfront: dim-block 0 n0 projections + first V tiles ----
        qk_unit(qT_t, wq_t, 0, 0)()
        qk_unit(kT_t, wk_t, 0, 0)()
        v_unit(0)()
        v_unit(2)()

        # filler queue, in need order
        queue.append(qk_unit(qT_t, wq_t, 0, HW))
        queue.append(qk_unit(kT_t, wk_t, 0, HW))
        queue.append(v_unit(4))
        queue.append(v_unit(6))
        for st in range(NKT // 2, NKT, 2):
            queue.append(v_unit(st))
        for j in range(1, NJ):
            queue.append(qk_unit(qT_t, wq_t, j, 0))
            queue.append(qk_unit(kT_t, wk_t, j, 0))
            queue.append(qk_unit(qT_t, wq_t, j, HW))
            queue.append(qk_unit(kT_t, wk_t, j, HW))

        for j in range(NPAIR):
            for half in range(2):
                attention_pair(j, half)
                if j == NPAIR - 1 and half == 0:
                    # all half-0 ctx normalized: queue its output proj
                    for st in range(NKT // 2):
                        queue.append(out_unit(st))
        fill(len(queue))
        for st in range(NKT // 2, NKT):
            out_unit(st)()

        if dbg is not None:
            nc.sync.dma_start(dbg["dqT"], qT_t[:])
            nc.sync.dma_start(dbg["dkT"], kT_t[:])
            nc.sync.dma_start(dbg["dv"], v_t[:])
            nc.sync.dma_start(dbg["dctxT"], ctxT_t[:])


def _compile():
    global _COMPILED
    if _COMPILED is not None:
        return _COMPILED
    nc = bacc.Bacc("TRN2", target_bir_lowering=False, debug=False,
                   num_devices=NCORES)
    aps = {
        "xT": nc.dram_tensor("xT", [D, S], BF16, kind="ExternalInput").ap(),
        "wq": nc.dram_tensor("wq", [D, DG], BF16, kind="ExternalInput").ap(),
        "wk": nc.dram_tensor("wk", [D, DG], BF16, kind="ExternalInput").ap(),
        "wv": nc.dram_tensor("wv", [D, DG], BF16, kind="ExternalInput").ap(),
        "wd": nc.dram_tensor("wd", [DG, D], BF16, kind="ExternalInput").ap(),
        "mask": nc.dram_tensor("mask", [P, P], BF16, kind="ExternalInput").ap(),
        "outp": nc.dram_tensor("outp", [S, D], F32, kind="ExternalOutput").ap(),
    }
    with tile.TileContext(nc) as tc:
        _build_body(tc, aps)
    nc.compile()
    _COMPILED = nc
    return nc


def _host_shards(x, w_qkv):
    """Per-core input dicts (bf16)."""
    xb = [np.ascontiguousarray(x[b].T).astype(NPBF16) for b in range(B)]
    mask = np.triu(np.ones((P, P), dtype=np.float32)).astype(NPBF16)
    w = w_qkv.reshape(D, H, 3, HD)  # col = h*192 + t*64 + d
    shards = []
    for c in range(NCORES):
        b, g = c // 2, c % 2
        hs = slice(g * GH, (g + 1) * GH)
        shards.append({
            "xT": xb[b],
            "wq": np.ascontiguousarray(
                w[:, hs, 0, :].reshape(D, DG)).astype(NPBF16),
            "wk": np.ascontiguousarray(
                w[:, hs, 1, :].reshape(D, DG)).astype(NPBF16),
            "wv": np.ascontiguousarray(
                w[:, hs, 2, :].reshape(D, DG)).astype(NPBF16),
            "wd": None,  # filled by caller (needs w_dense)
            "mask": mask,
        })
    return shards


def _reference_fallback(x, w_qkv, b_qkv, w_dense, b_dense):
    qkv = x @ w_qkv + b_qkv
    b, s, d = x.shape
    qkv = qkv.reshape(b, s, H, 3 * HD).transpose(0, 2, 1, 3)
    q, k, v = np.split(qkv, 3, axis=-1)
    scores = np.einsum("bhqd,bhkd->bhqk", q, k) / np.sqrt(HD)
    causal = np.tril(np.ones((s, s), dtype=bool))[None, None]
    scores = np.where(causal, scores, -10000.0)
    scores -= scores.max(axis=-1, keepdims=True)
    p = np.exp(scores)
    p /= p.sum(axis=-1, keepdims=True)
    ctx = np.einsum("bhqk,bhkd->bhqd", p, v)
    ctx = ctx.transpose(0, 2, 1, 3).reshape(b, s, d)
    return (ctx @ w_dense + b_dense).astype(np.float32)


def kernel(x, w_qkv, b_qkv, w_dense, b_dense, _want_trace=False):
    x = np.asarray(x, dtype=np.float32)
    w_qkv = np.asarray(w_qkv, dtype=np.float32)
    b_qkv = np.asarray(b_qkv, dtype=np.float32)
    w_dense = np.asarray(w_dense, dtype=np.float32)
    b_dense = np.asarray(b_dense, dtype=np.float32)

    if np.abs(b_qkv).max() > 0:
        # qkv bias is zero in the problem spec; general path for safety
        return _reference_fallback(x, w_qkv, b_qkv, w_dense, b_dense)

    nc = _compile()
    shards = _host_shards(x, w_qkv)
    for c in range(NCORES):
        g = c % 2
        shards[c]["wd"] = np.ascontiguousarray(
            w_dense[g * DG:(g + 1) * DG, :]).astype(NPBF16)

    res = bass_utils.run_bass_kernel_spmd(
        nc, shards, core_ids=list(range(NCORES)), trace=_want_trace,
    )
    out = np.empty((B, S, D), dtype=np.float32)
    for b in range(B):
        out[b] = res.results[2 * b]["outp"] + res.results[2 * b + 1]["outp"]
    out += b_dense[None, None, :]
    if _want_trace:
        return out, res
    return out


# revision 15
# speedup vs baseline: 1.0230x; 1.0230x over previous
"""GPT self-attention (B=4, S=2048, D=1024, H=16) on 8 NeuronCores.

Sharding: core c = (batch b = c//2, head-group g = c%2 of 8 heads).
Each core computes q/k/v projections for its 8 heads, causal attention,
and a partial output projection (rows of w_dense for its heads).
Host sums the two partials per batch (tensor-parallel unshard) + bias.

Schedule: heads are processed in pairs (2j, 2j+1) sharing one key-tile
loop, so the two heads' score matmuls (contraction 64, base partitions
0/64) run concurrently in disjoint PE row groups.  Projection GEMMs are
chopped into ~1.7us work units and drip-fed into the attention phases'
ACT-bound gaps.  Softmax normalization is deferred per pair: rowsums
(ones-column of V) are DMA-collected into a [16, S] tile, inverted with
one 2-lane approx-reciprocal per pair, broadcast on GpSimd, and applied
with one multiply per 64-row block.
"""

import numpy as np
import ml_dtypes

import concourse.bass as bass
import concourse.mybir as mybir
import concourse.tile as tile
from concourse import bacc
from concourse import bass_utils

B, S, D, H = 4, 2048, 1024, 16
HD = D // H          # 64
NCORES = 8
GH = 8               # heads per core (group)
DG = GH * HD         # 512 dims per group
P = 128
NKT = S // P         # 16 key tiles
NJ = DG // P         # 4 partition-tiles of group dims
NKD = D // P         # 8 contraction tiles for projections
CH = 512             # psum chunk (one bank of f32)
HW = 1024            # q-half width
NPAIR = GH // 2      # 4 head pairs

BF16 = mybir.dt.bfloat16
F32 = mybir.dt.float32
NPBF16 = ml_dtypes.bfloat16

_COMPILED = None


def _build_body(tc, aps, dbg=None):
    nc = tc.nc
    xT = aps["xT"].rearrange("(k p) s -> p k s", p=P)      # [128, 8, 2048]
    wq = aps["wq"].rearrange("(k p) m -> p k m", p=P)      # [128, 8, 512]
    wk = aps["wk"].rearrange("(k p) m -> p k m", p=P)
    wv = aps["wv"].rearrange("(k p) m -> p k m", p=P)
    wd = aps["wd"].rearrange("(j p) n -> p j n", p=P)      # [128, 4, 1024]
    maskin = aps["mask"]                                   # [128, 128] bf16
    outp = aps["outp"]                                     # [2048, 1024] f32

    Exp = mybir.ActivationFunctionType.Exp

    with (
        tc.tile_pool(name="const", bufs=1) as cpool,
        tc.tile_pool(name="pts", bufs=4) as ppool,
        tc.tile_pool(name="c64", bufs=3) as cstg,
        tc.tile_pool(name="rsg", bufs=2) as rstg,
        tc.tile_pool(name="bc", bufs=2) as bcp,
        tc.tile_pool(name="r0", bufs=2) as r0p,
        tc.tile_pool(name="ost", bufs=2) as ostg,
        tc.tile_pool(name="pssc", bufs=2, space=bass.MemorySpace.PSUM) as psc,
        tc.tile_pool(name="psctx", bufs=2, space=bass.MemorySpace.PSUM) as pcx,
    ):
        # ---- persistent SBUF tensors ----
        xT_t = cpool.tile([P, NKD, S], BF16, tag="xT")
        wq_t = cpool.tile([P, NKD, DG], BF16, tag="wq")
        wk_t = cpool.tile([P, NKD, DG], BF16, tag="wk")
        wv_t = cpool.tile([P, NKD, DG], BF16, tag="wv")
        wd_t = cpool.tile([P, NJ, D], BF16, tag="wd")
        mask_t = cpool.tile([P, P], BF16, tag="mask")
        qT_t = cpool.tile([P, NJ, S], BF16, tag="qT")      # [dim, s]
        kT_t = cpool.tile([P, NJ, S], BF16, tag="kT")
        # v_aug: per s-tile, per head: 64 v-dims + ones column (65 wide)
        v_t = cpool.tile([P, NKT, GH * (HD + 1)], BF16, tag="v")
        ctxT_t = cpool.tile([P, NJ, S], BF16, tag="ctxT")  # normalized ctx^T

        nc.sync.dma_start(wq_t[:], wq)
        nc.sync.dma_start(wk_t[:], wk)
        for kt in range(NKD):
            nc.sync.dma_start(xT_t[:, kt, :], xT[:, kt, :])
        nc.sync.dma_start(wv_t[:], wv)
        nc.sync.dma_start(mask_t[:], maskin)
        nc.sync.dma_start(wd_t[:], wd)
        # ones columns of v_aug
        v_heads = v_t.rearrange("p t (h c) -> p t h c", c=HD + 1)
        nc.vector.memset(v_heads[:, :, :, HD:], 1.0)

        # ---- projection work units (~1.7us of PE each) ----
        def qk_unit(dst, w, j, n0):
            def emit():
                ps = psc.tile([P, 2 * CH], F32, tag="sc")
                for sub in range(2):
                    for kt in range(NKD):
                        nc.tensor.matmul(
                            ps[:, sub * CH:(sub + 1) * CH],
                            w[:, kt, j * P:(j + 1) * P],
                            xT_t[:, kt, n0 + sub * CH:n0 + (sub + 1) * CH],
                            start=(kt == 0), stop=(kt == NKD - 1),
                        )
                nc.vector.tensor_copy(dst[:, j, n0:n0 + 2 * CH], ps[:])
            return emit

        def v_unit(st):
            def emit():
                ps = psc.tile([P, 2 * CH], F32, tag="sc")
                for sub in range(2):
                    for kt in range(NKD):
                        nc.tensor.matmul(
                            ps[:, sub * CH:(sub + 1) * CH],
                            xT_t[:, kt, (st + sub) * P:(st + sub + 1) * P],
                            wv_t[:, kt, :],
                            start=(kt == 0), stop=(kt == NKD - 1),
                        )
                for sub in range(2):
                    nc.vector.tensor_copy(
                        v_heads[:, st + sub, :, 0:HD],
                        ps[:, sub * CH:(sub + 1) * CH]
                        .rearrange("p (h c) -> p h c", c=HD)[:],
                    )
            return emit

        def out_unit(st):
            def emit():
                ps = psc.tile([P, 2 * CH], F32, tag="sc")
                for sub in range(2):
                    for j in range(NJ):
                        nc.tensor.matmul(
                            ps[:, sub * CH:(sub + 1) * CH],
                            ctxT_t[:, j, st * P:(st + 1) * P],
                            wd_t[:, j, sub * CH:(sub + 1) * CH],
                            start=(j == 0), stop=(j == NJ - 1),
                        )
                ost = ostg.tile([P, 2 * CH], F32, tag="ost")
                nc.vector.tensor_copy(ost[:], ps[:])
                nc.sync.dma_start(outp[st * P:(st + 1) * P, :], ost[:])
            return emit

        queue = []

        def fill(n=1):
            for _ in range(n):
                if queue:
                    queue.pop(0)()

        # ---- one head pair (2j, 2j+1) over one q-half ----
        def attention_pair(j, half, fills=()):
            lo, hi = HW * half, HW * (half + 1)
            hE, hO = 2 * j, 2 * j + 1
            ctxE = pcx.tile([HD + 1, HW], F32, tag="ctx")
            ctxO = pcx.tile([HD + 1, HW], F32, tag="ctx")
            nkt = (half + 1) * (NKT // 2)

            def emit_ctx(kt, q0, pe, po):
                chunks = [(max(q0, CH * m), CH * (m + 1))
                          for m in range(q0 // CH, hi // CH)]
                for h, ctxp, pts in ((hE, ctxE, pe), (hO, ctxO, po)):
                    for c0, c1 in chunks:
                        nc.tensor.matmul(
                            ctxp[:, c0 - lo:c1 - lo],
                            v_t[:, kt, h * (HD + 1):(h + 1) * (HD + 1)],
                            pts[:, c0 - q0:c1 - q0],
                            start=(kt == 0), stop=(kt == nkt - 1),
                            skip_group_check=True,
                        )

            pend = None
            for kt in range(nkt):
                q0 = max(P * kt, lo)
                width = hi - q0
                pe = ppool.tile([P, HW], BF16, tag="pts")
                po = ppool.tile([P, HW], BF16, tag="pts")
                for pb, pts in ((0, pe), (64, po)):
                    sps = psc.tile([P, 2 * CH], F32, tag="sc")
                    for c in range(0, width, CH):
                        cw = min(CH, width - c)
                        nc.tensor.matmul(
                            sps[:, c:c + cw],
                            kT_t[pb:pb + HD, j, P * kt:P * (kt + 1)],
                            qT_t[pb:pb + HD, j, q0 + c:q0 + c + cw],
                            start=True, stop=True,
                        )
                    nc.scalar.activation(
                        pts[:, 0:width], sps[:, 0:width], Exp,
                        scale=1.0 / np.sqrt(HD),
                    )
                    if q0 == P * kt:  # diagonal tile: causal mask
                        nc.vector.tensor_mul(pts[:, 0:P], pts[:, 0:P], mask_t[:])
                if kt in fills:
                    fill(1)
                if pend is not None:
                    emit_ctx(*pend)
                pend = (kt, q0, pe, po)
            emit_ctx(*pend)
            # ---- drain psum fast: ctx rows on DVE (bf16), rowsum on ACT ----
            work = []
            for h, ctxp, pb in ((hE, ctxE, 0), (hO, ctxO, 64)):
                c64 = cstg.tile([HD, HW], BF16, tag="c64")
                nc.vector.tensor_copy(c64[:], ctxp[0:HD, :])
                rsg = rstg.tile([HD + 1, HW], F32, tag="rsg")
                nc.vector.tensor_copy(rsg[HD:HD + 1, :], ctxp[HD:HD + 1, :])
                work.append((c64, rsg, pb))
            # ---- 1/rowsum -> broadcast -> normalize (off critical path) ----
            for c64, rsg, pb in work:
                r0 = r0p.tile([1, HW], F32, tag="r0")
                nc.sync.dma_start(r0[:], rsg[HD:HD + 1, :])
                nc.vector.reciprocal_approx_fast(r0[:], r0[:])
                bc = bcp.tile([HD, HW], F32, tag="bc")
                nc.gpsimd.partition_broadcast(bc[:], r0[:])
                if pb == 0:
                    nc.vector.tensor_mul(
                        ctxT_t[0:HD, j, lo:hi], c64[:], bc[:])
                else:
                    nc.vector.tensor_mul(c64[:], c64[:], bc[:])
                    nc.sync.dma_start(ctxT_t[pb:pb + P // 2, j, lo:hi], c64[:])

        # ---- PE warmup: junk matmuls on wq while xT loads, to flip the
        # HAM clock gate to 8/8 before the real work arrives ----
        junk = psc.tile([P, 2 * CH], F32, tag="sc")
        for i in range(24):
            nc.tensor.matmul(
                junk[:, 0:CH], wq_t[:, 0, 0:P], wq_t[:, i % NKD, 0:CH],
                start=True, stop=True, skip_group_check=True,
            )

        # ---- upfront: dim-block 0 n0 projections + first V tiles ----
        qk_unit(qT_t, wq_t, 0, 0)()
        qk_unit(kT_t, wk_t, 0, 0)()
        v_unit(0)()
        v_unit(2)()

        # filler queue, in need order
        queue.append(qk_unit(qT_t, wq_t, 0, HW))
        queue.append(qk_unit(kT_t, wk_t, 0, HW))
        queue.append(v_unit(4))
        queue.append(v_unit(6))
        for st in range(NKT // 2, NKT, 2):
            queue.append(v_unit(st))
        for j in range(1, NJ):
            queue.append(qk_unit(qT_t, wq_t, j, 0))
            queue.append(qk_unit(kT_t, wk_t, j, 0))
            queue.append(qk_unit(qT_t, wq_t, j, HW))
            queue.append(qk_unit(kT_t, wk_t, j, HW))

        # phase order: p3h0 early so half-0 output-proj work unblocks
        # in time to fill the late ACT-bound phases (HAM warmth)
        plan = [
            (0, 0, (1, 3, 5, 7)),
            (0, 1, (2, 4, 6, 8, 11, 14)),
            (1, 0, (3, 7)),
            (1, 1, (2, 5, 8, 11)),
            (2, 0, (3, 7)),
            (3, 0, (3, 7)),
            (2, 1, (2, 4, 6, 8, 11, 14)),
            (3, 1, (2, 5, 8)),
        ]
        for j, half, fills in plan:
            attention_pair(j, half, fills)
            if (j, half) == (3, 0):
                # all half-0 ctx normalized: queue its output proj
                for st in range(NKT // 2):
                    queue.append(out_unit(st))
        fill(len(queue))
        for st in range(NKT // 2, NKT):
            out_unit(st)()

        if dbg is not None:
            nc.sync.dma_start(dbg["dqT"], qT_t[:])
            nc.sync.dma_start(dbg["dkT"], kT_t[:])
            nc.sync.dma_start(dbg["dv"], v_t[:])
            nc.sync.dma_start(dbg["dctxT"], ctxT_t[:])


def _compile():
    global _COMPILED
    if _COMPILED is not None:
        return _COMPILED
    nc = bacc.Bacc("TRN2", target_bir_lowering=False, debug=False,
                   num_devices=NCORES)
    aps = {
        "xT": nc.dram_tensor("xT", [D, S], BF16, kind="ExternalInput").ap(),
        "wq": nc.dram_tensor("wq", [D, DG], BF16, kind="ExternalInput").ap(),
        "wk": nc.dram_tensor("wk", [D, DG], BF16, kind="ExternalInput").ap(),
        "wv": nc.dram_tensor("wv", [D, DG], BF16, kind="ExternalInput").ap(),
        "wd": nc.dram_tensor("wd", [DG, D], BF16, kind="ExternalInput").ap(),
        "mask": nc.dram_tensor("mask", [P, P], BF16, kind="ExternalInput").ap(),
        "outp": nc.dram_tensor("outp", [S, D], F32, kind="ExternalOutput").ap(),
    }
    with tile.TileContext(nc) as tc:
        _build_body(tc, aps)
    nc.compile()
    _COMPILED = nc
    return nc


def _host_shards(x, w_qkv):
    """Per-core input dicts (bf16)."""
    xb = [np.ascontiguousarray(x[b].T).astype(NPBF16) for b in range(B)]
    mask = np.triu(np.ones((P, P), dtype=np.float32)).astype(NPBF16)
    w = w_qkv.reshape(D, H, 3, HD)  # col = h*192 + t*64 + d
    shards = []
    for c in range(NCORES):
        b, g = c // 2, c % 2
        hs = slice(g * GH, (g + 1) * GH)
        shards.append({
            "xT": xb[b],
            "wq": np.ascontiguousarray(
                w[:, hs, 0, :].reshape(D, DG)).astype(NPBF16),
            "wk": np.ascontiguousarray(
                w[:, hs, 1, :].reshape(D, DG)).astype(NPBF16),
            "wv": np.ascontiguousarray(
                w[:, hs, 2, :].reshape(D, DG)).astype(NPBF16),
            "wd": None,  # filled by caller (needs w_dense)
            "mask": mask,
        })
    return shards


def _reference_fallback(x, w_qkv, b_qkv, w_dense, b_dense):
    qkv = x @ w_qkv + b_qkv
    b, s, d = x.shape
    qkv = qkv.reshape(b, s, H, 3 * HD).transpose(0, 2, 1, 3)
    q, k, v = np.split(qkv, 3, axis=-1)
    scores = np.einsum("bhqd,bhkd->bhqk", q, k) / np.sqrt(HD)
    causal = np.tril(np.ones((s, s), dtype=bool))[None, None]
    scores = np.where(causal, scores, -10000.0)
    scores -= scores.max(axis=-1, keepdims=True)
    p = np.exp(scores)
    p /= p.sum(axis=-1, keepdims=True)
    ctx = np.einsum("bhqk,bhkd->bhqd", p, v)
    ctx = ctx.transpose(0, 2, 1, 3).reshape(b, s, d)
    return (ctx @ w_dense + b_dense).astype(np.float32)


def kernel(x, w_qkv, b_qkv, w_dense, b_dense, _want_trace=False):
    x = np.asarray(x, dtype=np.float32)
    w_qkv = np.asarray(w_qkv, dtype=np.float32)
    b_qkv = np.asarray(b_qkv, dtype=np.float32)
    w_dense = np.asarray(w_dense, dtype=np.float32)
    b_dense = np.asarray(b_dense, dtype=np.float32)

    if np.abs(b_qkv).max() > 0:
        # qkv bias is zero in the problem spec; general path for safety
        return _reference_fallback(x, w_qkv, b_qkv, w_dense, b_dense)

    nc = _compile()
    shards = _host_shards(x, w_qkv)
    for c in range(NCORES):
        g = c % 2
        shards[c]["wd"] = np.ascontiguousarray(
            w_dense[g * DG:(g + 1) * DG, :]).astype(NPBF16)

    res = bass_utils.run_bass_kernel_spmd(
        nc, shards, core_ids=list(range(NCORES)), trace=_want_trace,
    )
    out = np.empty((B, S, D), dtype=np.float32)
    for b in range(B):
        out[b] = res.results[2 * b]["outp"] + res.results[2 * b + 1]["outp"]
    out += b_dense[None, None, :]
    if _want_trace:
        return out, res
    return out


# revision 17
# speedup vs baseline: 1.0335x; 1.0103x over previous
"""GPT self-attention (B=4, S=2048, D=1024, H=16) on 8 NeuronCores.

Sharding: core c = (batch b = c//2, head-group g = c%2 of 8 heads).
Each core computes q/k/v projections for its 8 heads, causal attention,
and a partial output projection (rows of w_dense for its heads).
Host sums the two partials per batch (tensor-parallel unshard) + bias.

Schedule: heads are processed in pairs (2j, 2j+1) sharing one key-tile
loop, so the two heads' score matmuls (contraction 64, base partitions
0/64) run concurrently in disjoint PE row groups.  Projection GEMMs are
chopped into ~1.7us work units and drip-fed into the attention phases'
ACT-bound gaps.  Softmax normalization is deferred per pair: rowsums
(ones-column of V) are DMA-collected into a [16, S] tile, inverted with
one 2-lane approx-reciprocal per pair, broadcast on GpSimd, and applied
with one multiply per 64-row block.
"""

import numpy as np
import ml_dtypes

import concourse.bass as bass
import concourse.mybir as mybir
import concourse.tile as tile
from concourse import bacc
from concourse import bass_utils

B, S, D, H = 4, 2048, 1024, 16
HD = D // H          # 64
NCORES = 8
GH = 8               # heads per core (group)
DG = GH * HD         # 512 dims per group
P = 128
NKT = S // P         # 16 key tiles
NJ = DG // P         # 4 partition-tiles of group dims
NKD = D // P         # 8 contraction tiles for projections
CH = 512             # psum chunk (one bank of f32)
HW = 1024            # q-half width
NPAIR = GH // 2      # 4 head pairs

BF16 = mybir.dt.bfloat16
F32 = mybir.dt.float32
NPBF16 = ml_dtypes.bfloat16

_COMPILED = None


def _build_body(tc, aps, dbg=None):
    nc = tc.nc
    xT = aps["xT"].rearrange("(k p) s -> p k s", p=P)      # [128, 8, 2048]
    wq = aps["wq"].rearrange("(k p) m -> p k m", p=P)      # [128, 8, 512]
    wk = aps["wk"].rearrange("(k p) m -> p k m", p=P)
    wv = aps["wv"].rearrange("(k p) m -> p k m", p=P)
    wd = aps["wd"].rearrange("(j p) n -> p j n", p=P)      # [128, 4, 1024]
    maskin = aps["mask"]                                   # [128, 128] bf16
    outp = aps["outp"]                                     # [2048, 1024] f32

    Exp = mybir.ActivationFunctionType.Exp

    with (
        tc.tile_pool(name="const", bufs=1) as cpool,
        tc.tile_pool(name="pts", bufs=4) as ppool,
        tc.tile_pool(name="c64", bufs=3) as cstg,
        tc.tile_pool(name="rsg", bufs=2) as rstg,
        tc.tile_pool(name="bc", bufs=2) as bcp,
        tc.tile_pool(name="r0", bufs=2) as r0p,
        tc.tile_pool(name="ost", bufs=2) as ostg,
        tc.tile_pool(name="pssc", bufs=2, space=bass.MemorySpace.PSUM) as psc,
        tc.tile_pool(name="psctx", bufs=2, space=bass.MemorySpace.PSUM) as pcx,
    ):
        # ---- persistent SBUF tensors ----
        xT_t = cpool.tile([P, NKD, S], BF16, tag="xT")
        wq_t = cpool.tile([P, NKD, DG], BF16, tag="wq")
        wk_t = cpool.tile([P, NKD, DG], BF16, tag="wk")
        wv_t = cpool.tile([P, NKD, DG], BF16, tag="wv")
        wd_t = cpool.tile([P, NJ, D], BF16, tag="wd")
        mask_t = cpool.tile([P, P], BF16, tag="mask")
        qT_t = cpool.tile([P, NJ, S], BF16, tag="qT")      # [dim, s]
        kT_t = cpool.tile([P, NJ, S], BF16, tag="kT")
        # v_aug: per s-tile, per head: 64 v-dims + ones column (65 wide)
        v_t = cpool.tile([P, NKT, GH * (HD + 1)], BF16, tag="v")
        ctxT_t = cpool.tile([P, NJ, S], BF16, tag="ctxT")  # normalized ctx^T

        nc.sync.dma_start(wq_t[:], wq)
        nc.sync.dma_start(wk_t[:], wk)
        for kt in range(NKD):
            nc.sync.dma_start(xT_t[:, kt, :], xT[:, kt, :])
        nc.sync.dma_start(wv_t[:], wv)
        nc.sync.dma_start(mask_t[:], maskin)
        nc.sync.dma_start(wd_t[:], wd)
        # ones columns of v_aug
        v_heads = v_t.rearrange("p t (h c) -> p t h c", c=HD + 1)
        nc.vector.memset(v_heads[:, :, :, HD:], 1.0)

        # ---- projection work units (~1.7us of PE each) ----
        def qk_unit(dst, w, j, n0):
            def emit():
                ps = psc.tile([P, 2 * CH], F32, tag="sc")
                for sub in range(2):
                    for kt in range(NKD):
                        nc.tensor.matmul(
                            ps[:, sub * CH:(sub + 1) * CH],
                            w[:, kt, j * P:(j + 1) * P],
                            xT_t[:, kt, n0 + sub * CH:n0 + (sub + 1) * CH],
                            start=(kt == 0), stop=(kt == NKD - 1),
                        )
                nc.vector.tensor_copy(dst[:, j, n0:n0 + 2 * CH], ps[:])
            return emit

        def v_unit(st):
            def emit():
                ps = psc.tile([P, 2 * CH], F32, tag="sc")
                for sub in range(2):
                    for kt in range(NKD):
                        nc.tensor.matmul(
                            ps[:, sub * CH:(sub + 1) * CH],
                            xT_t[:, kt, (st + sub) * P:(st + sub + 1) * P],
                            wv_t[:, kt, :],
                            start=(kt == 0), stop=(kt == NKD - 1),
                        )
                for sub in range(2):
                    nc.vector.tensor_copy(
                        v_heads[:, st + sub, :, 0:HD],
                        ps[:, sub * CH:(sub + 1) * CH]
                        .rearrange("p (h c) -> p h c", c=HD)[:],
                    )
            return emit

        def out_unit(st):
            def emit():
                ps = psc.tile([P, 2 * CH], F32, tag="sc")
                for sub in range(2):
                    for j in range(NJ):
                        nc.tensor.matmul(
                            ps[:, sub * CH:(sub + 1) * CH],
                            ctxT_t[:, j, st * P:(st + 1) * P],
                            wd_t[:, j, sub * CH:(sub + 1) * CH],
                            start=(j == 0), stop=(j == NJ - 1),
                        )
                ost = ostg.tile([P, 2 * CH], F32, tag="ost")
                nc.vector.tensor_copy(ost[:], ps[:])
                nc.sync.dma_start(outp[st * P:(st + 1) * P, :], ost[:])
            return emit

        queue = []

        def fill(n=1):
            for _ in range(n):
                if queue:
                    queue.pop(0)()

        # ---- one head pair (2j, 2j+1) over one q-half ----
        def attention_pair(j, half, fills=()):
            lo, hi = HW * half, HW * (half + 1)
            hE, hO = 2 * j, 2 * j + 1
            ctxE = pcx.tile([HD + 1, HW], F32, tag="ctx")
            ctxO = pcx.tile([HD + 1, HW], F32, tag="ctx")
            nkt = (half + 1) * (NKT // 2)

            def emit_ctx(kt, q0, pe, po):
                chunks = [(max(q0, CH * m), CH * (m + 1))
                          for m in range(q0 // CH, hi // CH)]
                for h, ctxp, pts in ((hE, ctxE, pe), (hO, ctxO, po)):
                    for c0, c1 in chunks:
                        nc.tensor.matmul(
                            ctxp[:, c0 - lo:c1 - lo],
                            v_t[:, kt, h * (HD + 1):(h + 1) * (HD + 1)],
                            pts[:, c0 - q0:c1 - q0],
                            start=(kt == 0), stop=(kt == nkt - 1),
                            skip_group_check=True,
                        )

            pend = None
            for kt in range(nkt):
                q0 = max(P * kt, lo)
                width = hi - q0
                pe = ppool.tile([P, HW], BF16, tag="pts")
                po = ppool.tile([P, HW], BF16, tag="pts")
                for pb, pts in ((0, pe), (64, po)):
                    sps = psc.tile([P, 2 * CH], F32, tag="sc")
                    for c in range(0, width, CH):
                        cw = min(CH, width - c)
                        nc.tensor.matmul(
                            sps[:, c:c + cw],
                            kT_t[pb:pb + HD, j, P * kt:P * (kt + 1)],
                            qT_t[pb:pb + HD, j, q0 + c:q0 + c + cw],
                            start=True, stop=True,
                        )
                    nc.scalar.activation(
                        pts[:, 0:width], sps[:, 0:width], Exp,
                        scale=1.0 / np.sqrt(HD),
                    )
                    if q0 == P * kt:  # diagonal tile: causal mask
                        nc.vector.tensor_mul(pts[:, 0:P], pts[:, 0:P], mask_t[:])
                if kt in fills:
                    fill(1)
                if pend is not None:
                    emit_ctx(*pend)
                pend = (kt, q0, pe, po)
            emit_ctx(*pend)
            # ---- drain each head's psum with ONE bf16 copy (rowsum row
            # included) so the ctx psum slots free as fast as possible ----
            work = []
            for h, ctxp, pb in ((hE, ctxE, 0), (hO, ctxO, 64)):
                c65 = cstg.tile([HD + 1, HW], BF16, tag="c64")
                nc.vector.tensor_copy(c65[:], ctxp[:])
                work.append((c65, pb))
            # ---- 1/rowsum -> broadcast -> normalize (off critical path) ----
            for c65, pb in work:
                r0b = r0p.tile([1, HW], BF16, tag="r0b")
                nc.sync.dma_start(r0b[:], c65[HD:HD + 1, :])
                r0 = r0p.tile([1, HW], F32, tag="r0")
                nc.vector.tensor_copy(r0[:], r0b[:])
                nc.vector.reciprocal_approx_fast(r0[:], r0[:])
                bc = bcp.tile([HD, HW], F32, tag="bc")
                nc.gpsimd.partition_broadcast(bc[:], r0[:])
                if pb == 0:
                    nc.vector.tensor_mul(
                        ctxT_t[0:HD, j, lo:hi], c65[0:HD, :], bc[:])
                else:
                    nc.vector.tensor_mul(c65[0:HD, :], c65[0:HD, :], bc[:])
                    nc.sync.dma_start(
                        ctxT_t[pb:pb + P // 2, j, lo:hi], c65[0:HD, :])

        # ---- PE warmup: junk matmuls on wq while xT loads, to flip the
        # HAM clock gate to 8/8 before the real work arrives ----
        junk = psc.tile([P, 2 * CH], F32, tag="sc")
        for i in range(24):
            nc.tensor.matmul(
                junk[:, 0:CH], wq_t[:, 0, 0:P], wq_t[:, i % NKD, 0:CH],
                start=True, stop=True, skip_group_check=True,
            )

        # ---- upfront: dim-block 0 n0 projections + first V tiles ----
        qk_unit(qT_t, wq_t, 0, 0)()
        qk_unit(kT_t, wk_t, 0, 0)()
        v_unit(0)()
        v_unit(2)()

        # filler queue, in need order
        queue.append(qk_unit(qT_t, wq_t, 0, HW))
        queue.append(qk_unit(kT_t, wk_t, 0, HW))
        queue.append(v_unit(4))
        queue.append(v_unit(6))
        for st in range(NKT // 2, NKT, 2):
            queue.append(v_unit(st))
        for j in range(1, NJ):
            queue.append(qk_unit(qT_t, wq_t, j, 0))
            queue.append(qk_unit(kT_t, wk_t, j, 0))
            queue.append(qk_unit(qT_t, wq_t, j, HW))
            queue.append(qk_unit(kT_t, wk_t, j, HW))

        # phase order: p3h0 early so half-0 output-proj work unblocks
        # in time to fill the late ACT-bound phases (HAM warmth)
        plan = [
            (0, 0, (1, 3, 5, 7)),
            (0, 1, (1, 3, 5, 8, 11, 14)),
            (1, 0, (1, 5)),
            (1, 1, (1, 5, 8, 11)),
            (2, 0, (1, 5)),
            (3, 0, (1, 5)),
            (2, 1, (1, 3, 5, 8, 11, 14)),
            (3, 1, (1, 3, 5)),
        ]
        for j, half, fills in plan:
            attention_pair(j, half, fills)
            if (j, half) == (3, 0):
                # all half-0 ctx normalized: queue its output proj
                for st in range(NKT // 2):
                    queue.append(out_unit(st))
        fill(len(queue))
        for st in range(NKT // 2, NKT):
            out_unit(st)()

        if dbg is not None:
            nc.sync.dma_start(dbg["dqT"], qT_t[:])
            nc.sync.dma_start(dbg["dkT"], kT_t[:])
            nc.sync.dma_start(dbg["dv"], v_t[:])
            nc.sync.dma_start(dbg["dctxT"], ctxT_t[:])


def _compile():
    global _COMPILED
    if _COMPILED is not None:
        return _COMPILED
    nc = bacc.Bacc("TRN2", target_bir_lowering=False, debug=False,
                   num_devices=NCORES)
    aps = {
        "xT": nc.dram_tensor("xT", [D, S], BF16, kind="ExternalInput").ap(),
        "wq": nc.dram_tensor("wq", [D, DG], BF16, kind="ExternalInput").ap(),
        "wk": nc.dram_tensor("wk", [D, DG], BF16, kind="ExternalInput").ap(),
        "wv": nc.dram_tensor("wv", [D, DG], BF16, kind="ExternalInput").ap(),
        "wd": nc.dram_tensor("wd", [DG, D], BF16, kind="ExternalInput").ap(),
        "mask": nc.dram_tensor("mask", [P, P], BF16, kind="ExternalInput").ap(),
        "outp": nc.dram_tensor("outp", [S, D], F32, kind="ExternalOutput").ap(),
    }
    with tile.TileContext(nc) as tc:
        _build_body(tc, aps)
    nc.compile()
    _COMPILED = nc
    return nc


def _host_shards(x, w_qkv):
    """Per-core input dicts (bf16)."""
    xb = [np.ascontiguousarray(x[b].T).astype(NPBF16) for b in range(B)]
    mask = np.triu(np.ones((P, P), dtype=np.float32)).astype(NPBF16)
    w = w_qkv.reshape(D, H, 3, HD)  # col = h*192 + t*64 + d
    shards = []
    for c in range(NCORES):
        b, g = c // 2, c % 2
        hs = slice(g * GH, (g + 1) * GH)
        shards.append({
            "xT": xb[b],
            "wq": np.ascontiguousarray(
                w[:, hs, 0, :].reshape(D, DG)).astype(NPBF16),
            "wk": np.ascontiguousarray(
                w[:, hs, 1, :].reshape(D, DG)).astype(NPBF16),
            "wv": np.ascontiguousarray(
                w[:, hs, 2, :].reshape(D, DG)).astype(NPBF16),
            "wd": None,  # filled by caller (needs w_dense)
            "mask": mask,
        })
    return shards


def _reference_fallback(x, w_qkv, b_qkv, w_dense, b_dense):
    qkv = x @ w_qkv + b_qkv
    b, s, d = x.shape
    qkv = qkv.reshape(b, s, H, 3 * HD).transpose(0, 2, 1, 3)
    q, k, v = np.split(qkv, 3, axis=-1)
    scores = np.einsum("bhqd,bhkd->bhqk", q, k) / np.sqrt(HD)
    causal = np.tril(np.ones((s, s), dtype=bool))[None, None]
    scores = np.where(causal, scores, -10000.0)
    scores -= scores.max(axis=-1, keepdims=True)
    p = np.exp(scores)
    p /= p.sum(axis=-1, keepdims=True)
    ctx = np.einsum("bhqk,bhkd->bhqd", p, v)
    ctx = ctx.transpose(0, 2, 1, 3).reshape(b, s, d)
    return (ctx @ w_dense + b_dense).astype(np.float32)


def kernel(x, w_qkv, b_qkv, w_dense, b_dense, _want_trace=False):
    x = np.asarray(x, dtype=np.float32)
    w_qkv = np.asarray(w_qkv, dtype=np.float32)
    b_qkv = np.asarray(b_qkv, dtype=np.float32)
    w_dense = np.asarray(w_dense, dtype=np.float32)
    b_dense = np.asarray(b_dense, dtype=np.float32)

    if np.abs(b_qkv).max() > 0:
        # qkv bias is zero in the problem spec; general path for safety
        return _reference_fallback(x, w_qkv, b_qkv, w_dense, b_dense)

    nc = _compile()
    shards = _host_shards(x, w_qkv)
    for c in range(NCORES):
        g = c % 2
        shards[c]["wd"] = np.ascontiguousarray(
            w_dense[g * DG:(g + 1) * DG, :]).astype(NPBF16)

    res = bass_utils.run_bass_kernel_spmd(
        nc, shards, core_ids=list(range(NCORES)), trace=_want_trace,
    )
    out = np.empty((B, S, D), dtype=np.float32)
    for b in range(B):
        out[b] = res.results[2 * b]["outp"] + res.results[2 * b + 1]["outp"]
    out += b_dense[None, None, :]
    if _want_trace:
        return out, res
    return out
